# revision 15
# baseline (speedup 1.0000x reference)
"""Trainium2 Bass kernel for nn_Custom_Loss_84937273246180.

reference:
    path = argmax(solution_matrix, axis=0)        # [8192] int
    nxt  = roll(path, -1)
    out  = sum(cost_matrix[path, nxt])            # [1] f32

Strategy (8 NeuronCores, two SPMD launches):
  Launch A (8 cores, column-sharded): core i owns columns [i*1024, (i+1)*1024)
    of solution_matrix and computes the full argmax over the 8192 rows for its
    columns. Row-fold via DVE max tree, cross-partition max via gpsimd
    partition_all_reduce, index extraction via is_equal mask + PE matmul with
    (p, t) index vectors. Outputs path_shard [1024] int32 per core.
    Host concatenates the 8 shards (pure unshard).
  Launch B (8 cores, term-sharded): core i owns gather terms
    [i*1024, (i+1)*1024). Inputs: full cost_matrix (replicated) and the
    host-sliced (path[j], path[j+1 mod n]) index pairs for its terms
    (pure sharding glue). Each term's value is fetched with a block
    indirect-DMA: one 64-element aligned block per (partition, term)
    containing cost[path[j], nxt[j]]; the wanted element is selected
    on-chip (iota==sel mask) and everything is reduced to one scalar via
    DVE reduce + a ones-vector PE matmul across partitions. Host sums the
    8 per-core partials (unshard).
"""

import numpy as np
from contextlib import ExitStack

import concourse.bass as bass
import concourse.bacc as bacc
import concourse.tile as tile
from concourse import mybir
from concourse import bass_isa
from concourse.bass_utils import run_bass_kernel_spmd

N = 8192
NCORES = 8
CPC = N // NCORES        # columns / gather-terms per core = 1024
CHUNK = 512              # columns processed per chunk (launch A)
NCHUNK = CPC // CHUNK    # 2
NTILES = N // 128        # 64 row tiles
NSUB = 8                 # sub-tiles (each holds 8 row tiles)
G = CPC // 128           # gather terms per partition (launch B) = 8
B = 64                   # gather block width (elements per indirect DMA row)

F32 = mybir.dt.float32
BF16 = mybir.dt.bfloat16
I32 = mybir.dt.int32

_cache = {}


# ---------------- Launch A: column-sharded argmax ----------------

def _build_argmax_nc(n_iters: int = 1, data_bufs: int = 8, mode: str = "full"):
    """SPMD kernel: input sol [NCHUNK, 8192, CHUNK] (one column shard,
    chunk-major so every DMA is a fully contiguous block), output
    path_shard [1024] int32 = argmax over axis 0 (rows).

    data_bufs > 8 gives the DMA stream lookahead across chunk (and For_i
    iteration) boundaries so loads overlap the fold/mask phases.

    n_iters > 1 wraps the body in a For_i hardware loop (used only for
    timing; instruction count stays constant)."""
    nc = bacc.Bacc("TRN2", target_bir_lowering=False, debug=False,
                   num_devices=NCORES)
    sol = nc.dram_tensor("sol", [NCHUNK, N, CHUNK], F32, kind="ExternalInput")
    path_out = nc.dram_tensor("path_shard", [CPC], I32, kind="ExternalOutput")

    # [chunk, sub, p, a, c]: row = (s*NSUB + a)*128 + p
    sol_v = sol.rearrange("h (s a p) c -> h s p a c", p=128, a=NSUB)

    with tile.TileContext(nc) as tc:
        with ExitStack() as ctx:
            data_pool = ctx.enter_context(
                tc.tile_pool(name="data", bufs=data_bufs))
            scratch_pool = ctx.enter_context(tc.tile_pool(name="scratch", bufs=1))
            mask_pool = ctx.enter_context(tc.tile_pool(name="mask", bufs=3))
            acc_pool = ctx.enter_context(tc.tile_pool(name="acc", bufs=1))
            small_pool = ctx.enter_context(tc.tile_pool(name="small", bufs=2))
            const_pool = ctx.enter_context(tc.tile_pool(name="const", bufs=1))
            psum_pool = ctx.enter_context(
                tc.tile_pool(name="psum", bufs=2, space="PSUM"))

            # lhsT index vectors: [128, t, 0] = p, [128, t, 1] = 128*t
            lt_i = const_pool.tile([128, NTILES, 2], I32)
            nc.gpsimd.iota(lt_i[:, :, 0], pattern=[[0, NTILES]], base=0,
                           channel_multiplier=1)
            nc.gpsimd.iota(lt_i[:, :, 1], pattern=[[128, NTILES]], base=0,
                           channel_multiplier=0)
            lt = const_pool.tile([128, NTILES, 2], BF16)
            nc.vector.tensor_copy(lt[:], lt_i[:])

            import contextlib
            loop_cm = (tc.For_i(0, n_iters, 1) if n_iters > 1
                       else contextlib.nullcontext())
            with loop_cm:
                for chunk in range(NCHUNK):
                    subs = []
                    msub = []
                    if mode == "dmaonly":
                        for s in range(NSUB):
                            st = data_pool.tile([128, NSUB, CHUNK], F32,
                                                tag="sub")
                            nc.sync.dma_start(out=st[:], in_=sol_v[chunk, s])
                        rowi = small_pool.tile([1, CHUNK], I32, tag="rowi")
                        nc.vector.memset(rowi[:], 0)
                        nc.sync.dma_start(
                            out=path_out[chunk * CHUNK:(chunk + 1) * CHUNK],
                            in_=rowi[0:1, :])
                        continue
                    for s in range(NSUB):
                        st = data_pool.tile([128, NSUB, CHUNK], F32, tag="sub")
                        nc.sync.dma_start(out=st[:], in_=sol_v[chunk, s])
                        subs.append(st)
                        # per-sub-tile fold: 8 row tiles -> 1 (tree, no chain)
                        t1 = scratch_pool.tile([128, 4, CHUNK], F32, tag="t1")
                        nc.vector.tensor_tensor(
                            out=t1[:], in0=st[:, 0:4, :], in1=st[:, 4:8, :],
                            op=mybir.AluOpType.max)
                        t2 = scratch_pool.tile([128, 2, CHUNK], F32, tag="t2")
                        nc.vector.tensor_tensor(
                            out=t2[:], in0=t1[:, 0:2, :], in1=t1[:, 2:4, :],
                            op=mybir.AluOpType.max)
                        # running max accumulator (2KB instead of a 16KB
                        # 8-slot buffer): acc = max(acc, sub_max)
                        ms = acc_pool.tile([128, 2, CHUNK], F32, tag="ms")
                        if s == 0:
                            msub = [ms]
                            nc.vector.tensor_tensor(
                                out=ms[:, 0, :], in0=t2[:, 0, :],
                                in1=t2[:, 1, :], op=mybir.AluOpType.max)
                        else:
                            ms = msub[0]
                            nc.vector.tensor_tensor(
                                out=ms[:, 1, :], in0=t2[:, 0, :],
                                in1=t2[:, 1, :], op=mybir.AluOpType.max)
                            nc.vector.tensor_tensor(
                                out=ms[:, 0, :], in0=ms[:, 0, :],
                                in1=ms[:, 1, :], op=mybir.AluOpType.max)
                    ms = msub[0]

                    # column max replicated across partitions
                    bmax = small_pool.tile([128, CHUNK], F32, tag="bmax")
                    if mode == "nopar":
                        # timing-only variant: results are wrong
                        nc.vector.tensor_copy(bmax[:], ms[:, 0, :])
                    else:
                        nc.gpsimd.partition_all_reduce(
                            bmax[:], ms[:, 0, :], channels=128,
                            reduce_op=bass_isa.ReduceOp.max)

                    # cmp + index matmuls
                    ps = psum_pool.tile([2, CHUNK], F32, tag="ps")
                    if mode != "nomask":
                        for s in range(NSUB):
                            st = subs[s]
                            mask = mask_pool.tile([128, NSUB, CHUNK], BF16,
                                                  tag="mask")
                            bmax_b = bass.AP(
                                tensor=bmax.tensor,
                                offset=bmax[:].offset,
                                ap=[bmax[:].ap[0], [0, NSUB], bmax[:].ap[1]],
                            )
                            nc.vector.tensor_tensor(
                                out=mask[:], in0=st[:], in1=bmax_b,
                                op=mybir.AluOpType.is_equal)
                            for j in range(NSUB):
                                t = s * NSUB + j
                                nc.tensor.matmul(
                                    ps[:], lt[:, t, :], mask[:, j, :],
                                    start=(t == 0), stop=(t == NTILES - 1))
                    else:
                        nc.tensor.matmul(
                            ps[:], lt[:, 0, :], bmax[:].bitcast(BF16)[:, 0:CHUNK],
                            start=True, stop=True)

                    # row = (128*t) + p ; psum row0 = sum p*mask, row1 = sum 128t*mask
                    sb2 = small_pool.tile([2, CHUNK], F32, tag="sb2")
                    nc.vector.tensor_copy(sb2[:], ps[:])
                    sbt = small_pool.tile([1, CHUNK], F32, tag="sbt")
                    nc.sync.dma_start(out=sbt[:], in_=sb2[1:2, :])
                    rowf = small_pool.tile([1, CHUNK], F32, tag="rowf")
                    nc.vector.tensor_tensor(
                        out=rowf[:], in0=sbt[:], in1=sb2[0:1, :],
                        op=mybir.AluOpType.add)
                    nc.vector.tensor_scalar(
                        out=rowf[:], in0=rowf[:], scalar1=float(N - 1),
                        scalar2=0.0, op0=mybir.AluOpType.min,
                        op1=mybir.AluOpType.max)
                    rowi = small_pool.tile([1, CHUNK], I32, tag="rowi")
                    nc.vector.tensor_copy(rowi[:], rowf[:])
                    nc.sync.dma_start(
                        out=path_out[chunk * CHUNK:(chunk + 1) * CHUNK],
                        in_=rowi[0:1, :])

    nc.compile()
    return nc


def _get_argmax_nc(n_iters: int = 1, data_bufs: int = 8, mode: str = "full"):
    key = ("argmax", n_iters, data_bufs, mode)
    if key not in _cache:
        _cache[key] = _build_argmax_nc(n_iters, data_bufs, mode)
    return _cache[key]


def argmax_in_maps(solution_matrix: np.ndarray):
    sol = np.ascontiguousarray(solution_matrix)
    in_maps = []
    for i in range(NCORES):
        shard = sol[:, i * CPC:(i + 1) * CPC]           # [8192, 1024]
        shard = shard.reshape(N, NCHUNK, CHUNK)          # [8192, 2, 512]
        shard = np.ascontiguousarray(shard.transpose(1, 0, 2))  # [2, 8192, 512]
        in_maps.append({"sol": shard})
    return in_maps


def run_argmax(solution_matrix: np.ndarray, n_iters: int = 1) -> np.ndarray:
    nc = _get_argmax_nc(n_iters)
    res = run_bass_kernel_spmd(nc, argmax_in_maps(solution_matrix),
                               core_ids=list(range(NCORES)))
    path = np.concatenate([res.results[i]["path_shard"] for i in range(NCORES)])
    return path.astype(np.int32)


# ---------------- Launch B: term-sharded gather + sum ----------------

def _build_gather_nc(n_iters: int = 1):
    """SPMD kernel (8 cores): inputs cost [8192, 8192] f32 (replicated),
    pt [1024] i32 = path[j] and nx [1024] i32 = path[(j+1) % n] for this
    core's terms j; output out [1] f32 = sum_j cost[pt[j], nx[j]].

    Terms live at (p, g), j = p*G + g. Each (p, g) fetches the 64-element
    aligned block of row pt containing column nx via one indirect DMA per
    g (base offset = (pt << 13) | (nx & ~63), one descriptor per
    partition), then selects element (nx & 63) with an iota==sel mask and
    reduces everything to a scalar."""
    nc = bacc.Bacc("TRN2", target_bir_lowering=False, debug=False,
                   num_devices=NCORES)
    cost = nc.dram_tensor("cost", [N, N], F32, kind="ExternalInput")
    pt_in = nc.dram_tensor("pt", [CPC], I32, kind="ExternalInput")
    nx_in = nc.dram_tensor("nx", [CPC], I32, kind="ExternalInput")
    out = nc.dram_tensor("out", [1], F32, kind="ExternalOutput")

    cost1 = cost.rearrange("r (k e) -> (r k) e", e=1)  # [N*N, 1]

    with tile.TileContext(nc) as tc:
        with ExitStack() as ctx:
            pool = ctx.enter_context(tc.tile_pool(name="p", bufs=2))
            const_pool = ctx.enter_context(tc.tile_pool(name="c", bufs=1))
            psum_pool = ctx.enter_context(
                tc.tile_pool(name="ps", bufs=2, space="PSUM"))

            # constants: iota [128, B] f32 (0..63 per partition), ones [128,1]
            io_i = const_pool.tile([128, B], I32)
            nc.gpsimd.iota(io_i[:], pattern=[[1, B]], base=0,
                           channel_multiplier=0)
            io_f = const_pool.tile([128, B], F32)
            nc.vector.tensor_copy(io_f[:], io_i[:])
            ones = const_pool.tile([128, 1], F32)
            nc.vector.memset(ones[:], 1.0)

            import contextlib
            loop_cm = (tc.For_i(0, n_iters, 1) if n_iters > 1
                       else contextlib.nullcontext())
            with loop_cm:
                pt = pool.tile([128, G], I32, tag="pt")
                nc.sync.dma_start(
                    out=pt[:], in_=pt_in.rearrange("(p g) -> p g", g=G))
                nx = pool.tile([128, G], I32, tag="nx")
                nc.sync.dma_start(
                    out=nx[:], in_=nx_in.rearrange("(p g) -> p g", g=G))

                # base = (pt << 13) | (nx & ~63); sel = nx & 63
                base = pool.tile([128, G], I32, tag="base")
                nc.vector.tensor_scalar(
                    out=base[:], in0=pt[:], scalar1=13, scalar2=None,
                    op0=mybir.AluOpType.logical_shift_left)
                hi = pool.tile([128, G], I32, tag="hi")
                nc.vector.tensor_scalar(
                    out=hi[:], in0=nx[:], scalar1=N - B, scalar2=None,
                    op0=mybir.AluOpType.bitwise_and)
                nc.vector.tensor_tensor(
                    out=base[:], in0=base[:], in1=hi[:],
                    op=mybir.AluOpType.bitwise_or)
                sel_i = pool.tile([128, G], I32, tag="sel_i")
                nc.vector.tensor_scalar(
                    out=sel_i[:], in0=nx[:], scalar1=B - 1, scalar2=None,
                    op0=mybir.AluOpType.bitwise_and)
                sel_f = pool.tile([128, G], F32, tag="sel_f")
                nc.vector.tensor_copy(sel_f[:], sel_i[:])

                # block gathers: blk[p, g, :] = cost1[base[p, g] .. +B-1]
                blk = pool.tile([128, G, B], F32, tag="blk")
                for g in range(G):
                    nc.gpsimd.indirect_dma_start(
                        out=blk[:, g, :], out_offset=None,
                        in_=cost1[:, :],
                        in_offset=bass.IndirectOffsetOnAxis(
                            ap=base[:, g:g + 1], axis=0))

                # m = (iota == sel); s1 = sum(m * blk) per partition
                m = pool.tile([128, G, B], F32, tag="m")
                io_b = bass.AP(
                    tensor=io_f.tensor, offset=io_f[:].offset,
                    ap=[io_f[:].ap[0], [0, G], io_f[:].ap[1]])
                sel_b = bass.AP(
                    tensor=sel_f.tensor, offset=sel_f[:].offset,
                    ap=[sel_f[:].ap[0], sel_f[:].ap[1], [0, B]])
                nc.vector.tensor_tensor(
                    out=m[:], in0=io_b, in1=sel_b,
                    op=mybir.AluOpType.is_equal)
                scr = pool.tile([128, G, B], F32, tag="scr")
                nc.vector.tensor_tensor(
                    out=scr[:], in0=m[:], in1=blk[:],
                    op=mybir.AluOpType.mult)
                s1 = pool.tile([128, 1], F32, tag="s1")
                nc.vector.reduce_sum(s1[:], scr[:],
                                     axis=mybir.AxisListType.XY)

                # cross-partition sum via ones-vector matmul
                pss = psum_pool.tile([1, 1], F32, tag="pss")
                nc.tensor.matmul(pss[:], ones[:], s1[:], start=True, stop=True)
                so = pool.tile([1, 1], F32, tag="so")
                nc.vector.tensor_copy(so[:], pss[:])
                nc.sync.dma_start(out=out[0:1], in_=so[0:1, 0])

    nc.compile()
    return nc


def _get_gather_nc(n_iters: int = 1):
    key = ("gather", n_iters)
    if key not in _cache:
        _cache[key] = _build_gather_nc(n_iters)
    return _cache[key]


def gather_in_maps(cost_matrix: np.ndarray, path: np.ndarray):
    cost = np.ascontiguousarray(cost_matrix)
    path = np.ascontiguousarray(path.astype(np.int32))
    nxt = np.roll(path, -1)
    return [{"cost": cost,
             "pt": path[i * CPC:(i + 1) * CPC],
             "nx": nxt[i * CPC:(i + 1) * CPC]}
            for i in range(NCORES)]


def run_gather(cost_matrix: np.ndarray, path: np.ndarray,
               n_iters: int = 1) -> np.ndarray:
    nc = _get_gather_nc(n_iters)
    res = run_bass_kernel_spmd(
        nc, gather_in_maps(cost_matrix, path),
        core_ids=list(range(NCORES)))
    total = np.float32(0.0)
    for i in range(NCORES):
        total += np.asarray(res.results[i]["out"], dtype=np.float32)[0]
    return np.asarray([total], dtype=np.float32)


def kernel(solution_matrix: np.ndarray, cost_matrix: np.ndarray) -> np.ndarray:
    path = run_argmax(solution_matrix)
    cost = run_gather(cost_matrix, path)
    return cost


if __name__ == "__main__":
    rng = np.random.default_rng(0)
    sol = rng.standard_normal((N, N), dtype=np.float32)
    cm = rng.random((N, N), dtype=np.float32)
    path = run_argmax(sol)
    want = sol.argmax(axis=0)
    print("argmax match:", np.array_equal(path, want),
          (path != want).sum(), "mismatches")
    got = run_gather(cm, path)
    exp = cm[want, np.roll(want, -1)].sum()
    print("gather:", got, "expect:", exp)


# revision 21
# speedup vs baseline: 1.1539x; 1.1539x over previous
"""Trainium2 Bass kernel for nn_Custom_Loss_84937273246180.

reference:
    path = argmax(solution_matrix, axis=0)        # [8192] int
    nxt  = roll(path, -1)
    out  = sum(cost_matrix[path, nxt])            # [1] f32

Strategy (8 NeuronCores, two SPMD launches):
  Launch A (8 cores, column-sharded): core i owns columns [i*1024, (i+1)*1024)
    of solution_matrix and computes the full argmax over the 8192 rows for its
    columns. Row-fold via DVE max tree, cross-partition max via gpsimd
    partition_all_reduce, index extraction via is_equal mask + PE matmul with
    (p, t) index vectors. Outputs path_shard [1024] int32 per core.
    Host concatenates the 8 shards (pure unshard).
  Launch B (8 cores, term-sharded): core i owns gather terms
    [i*1024, (i+1)*1024). Inputs: full cost_matrix (replicated) and the
    host-sliced (path[j], path[j+1 mod n]) index pairs for its terms
    (pure sharding glue). Each term's value is fetched with a block
    indirect-DMA: one 64-element aligned block per (partition, term)
    containing cost[path[j], nxt[j]]; the wanted element is selected
    on-chip (iota==sel mask) and everything is reduced to one scalar via
    DVE reduce + a ones-vector PE matmul across partitions. Host sums the
    8 per-core partials (unshard).
"""

import numpy as np
from contextlib import ExitStack

import concourse.bass as bass
import concourse.bacc as bacc
import concourse.tile as tile
from concourse import mybir
from concourse import bass_isa
from concourse.bass_utils import run_bass_kernel_spmd

N = 8192
NCORES = 8
CPC = N // NCORES        # columns / gather-terms per core = 1024
CHUNK = 256              # columns processed per chunk (launch A)
NCHUNK = CPC // CHUNK    # 4
DATA_BUFS = 16           # 2x the sub-tiles per chunk: cross-chunk overlap
NTILES = N // 128        # 64 row tiles
NSUB = 8                 # sub-tiles (each holds 8 row tiles)
G = CPC // 128           # gather terms per partition (launch B) = 8
B = 64                   # gather block width (elements per indirect DMA row)

F32 = mybir.dt.float32
BF16 = mybir.dt.bfloat16
I32 = mybir.dt.int32

_cache = {}


# ---------------- Launch A: column-sharded argmax ----------------

def _build_argmax_nc(n_iters: int = 1, data_bufs: int = DATA_BUFS,
                     mode: str = "full", chunk_w: int = CHUNK):
    """SPMD kernel: input sol [nchunk, 8192, chunk_w] (one column shard,
    chunk-major so every DMA is a fully contiguous block), output
    path_shard [1024] int32 = argmax over axis 0 (rows).

    data_bufs = 2 * (number of sub-tiles per chunk) double-buffers the DMA
    stream across chunk (and For_i iteration) boundaries so loads overlap
    the fold/mask phases of the previous chunk.

    n_iters > 1 wraps the body in a For_i hardware loop (used only for
    timing; instruction count stays constant)."""
    nchunk = CPC // chunk_w
    nc = bacc.Bacc("TRN2", target_bir_lowering=False, debug=False,
                   num_devices=NCORES)
    sol = nc.dram_tensor("sol", [nchunk, N, chunk_w], F32,
                         kind="ExternalInput")
    path_out = nc.dram_tensor("path_shard", [CPC], I32, kind="ExternalOutput")

    # [chunk, sub, p, a, c]: row = (s*NSUB + a)*128 + p
    sol_v = sol.rearrange("h (s a p) c -> h s p a c", p=128, a=NSUB)

    with tile.TileContext(nc) as tc:
        with ExitStack() as ctx:
            data_pool = ctx.enter_context(
                tc.tile_pool(name="data", bufs=data_bufs))
            scratch_pool = ctx.enter_context(tc.tile_pool(name="scratch", bufs=1))
            mask_pool = ctx.enter_context(tc.tile_pool(name="mask", bufs=3))
            acc_pool = ctx.enter_context(tc.tile_pool(name="acc", bufs=1))
            small_pool = ctx.enter_context(tc.tile_pool(name="small", bufs=2))
            const_pool = ctx.enter_context(tc.tile_pool(name="const", bufs=1))
            psum_pool = ctx.enter_context(
                tc.tile_pool(name="psum", bufs=2, space="PSUM"))

            # lhsT index vectors: [128, t, 0] = p, [128, t, 1] = 128*t
            lt_i = const_pool.tile([128, NTILES, 2], I32)
            nc.gpsimd.iota(lt_i[:, :, 0], pattern=[[0, NTILES]], base=0,
                           channel_multiplier=1)
            nc.gpsimd.iota(lt_i[:, :, 1], pattern=[[128, NTILES]], base=0,
                           channel_multiplier=0)
            lt = const_pool.tile([128, NTILES, 2], BF16)
            nc.vector.tensor_copy(lt[:], lt_i[:])

            import contextlib
            loop_cm = (tc.For_i(0, n_iters, 1) if n_iters > 1
                       else contextlib.nullcontext())
            with loop_cm:
                for chunk in range(nchunk):
                    subs = []
                    msub = []
                    if mode == "dmaonly":
                        for s in range(NSUB):
                            st = data_pool.tile([128, NSUB, chunk_w], F32,
                                                tag="sub")
                            nc.sync.dma_start(out=st[:], in_=sol_v[chunk, s])
                        rowi = small_pool.tile([1, chunk_w], I32, tag="rowi")
                        nc.vector.memset(rowi[:], 0)
                        nc.sync.dma_start(
                            out=path_out[chunk * chunk_w:(chunk + 1) * chunk_w],
                            in_=rowi[0:1, :])
                        continue
                    for s in range(NSUB):
                        st = data_pool.tile([128, NSUB, chunk_w], F32, tag="sub")
                        nc.sync.dma_start(out=st[:], in_=sol_v[chunk, s])
                        subs.append(st)
                        # per-sub-tile fold: 8 row tiles -> 1 (tree, no chain)
                        t1 = scratch_pool.tile([128, 4, chunk_w], F32, tag="t1")
                        nc.vector.tensor_tensor(
                            out=t1[:], in0=st[:, 0:4, :], in1=st[:, 4:8, :],
                            op=mybir.AluOpType.max)
                        t2 = scratch_pool.tile([128, 2, chunk_w], F32, tag="t2")
                        nc.vector.tensor_tensor(
                            out=t2[:], in0=t1[:, 0:2, :], in1=t1[:, 2:4, :],
                            op=mybir.AluOpType.max)
                        # running max accumulator (2KB instead of a 16KB
                        # 8-slot buffer): acc = max(acc, sub_max)
                        ms = acc_pool.tile([128, 2, chunk_w], F32, tag="ms")
                        if s == 0:
                            msub = [ms]
                            nc.vector.tensor_tensor(
                                out=ms[:, 0, :], in0=t2[:, 0, :],
                                in1=t2[:, 1, :], op=mybir.AluOpType.max)
                        else:
                            ms = msub[0]
                            nc.vector.tensor_tensor(
                                out=ms[:, 1, :], in0=t2[:, 0, :],
                                in1=t2[:, 1, :], op=mybir.AluOpType.max)
                            nc.vector.tensor_tensor(
                                out=ms[:, 0, :], in0=ms[:, 0, :],
                                in1=ms[:, 1, :], op=mybir.AluOpType.max)
                    ms = msub[0]

                    # column max replicated across partitions
                    bmax = small_pool.tile([128, chunk_w], F32, tag="bmax")
                    if mode == "nopar":
                        # timing-only variant: results are wrong
                        nc.vector.tensor_copy(bmax[:], ms[:, 0, :])
                    else:
                        nc.gpsimd.partition_all_reduce(
                            bmax[:], ms[:, 0, :], channels=128,
                            reduce_op=bass_isa.ReduceOp.max)

                    # cmp + index matmuls
                    ps = psum_pool.tile([2, chunk_w], F32, tag="ps")
                    if mode != "nomask":
                        for s in range(NSUB):
                            st = subs[s]
                            mask = mask_pool.tile([128, NSUB, chunk_w], BF16,
                                                  tag="mask")
                            bmax_b = bass.AP(
                                tensor=bmax.tensor,
                                offset=bmax[:].offset,
                                ap=[bmax[:].ap[0], [0, NSUB], bmax[:].ap[1]],
                            )
                            nc.vector.tensor_tensor(
                                out=mask[:], in0=st[:], in1=bmax_b,
                                op=mybir.AluOpType.is_equal)
                            for j in range(NSUB):
                                t = s * NSUB + j
                                nc.tensor.matmul(
                                    ps[:], lt[:, t, :], mask[:, j, :],
                                    start=(t == 0), stop=(t == NTILES - 1))
                    else:
                        nc.tensor.matmul(
                            ps[:], lt[:, 0, :], bmax[:].bitcast(BF16)[:, 0:chunk_w],
                            start=True, stop=True)

                    # row = (128*t) + p ; psum row0 = sum p*mask, row1 = sum 128t*mask
                    sb2 = small_pool.tile([2, chunk_w], F32, tag="sb2")
                    nc.vector.tensor_copy(sb2[:], ps[:])
                    sbt = small_pool.tile([1, chunk_w], F32, tag="sbt")
                    nc.sync.dma_start(out=sbt[:], in_=sb2[1:2, :])
                    rowf = small_pool.tile([1, chunk_w], F32, tag="rowf")
                    nc.vector.tensor_tensor(
                        out=rowf[:], in0=sbt[:], in1=sb2[0:1, :],
                        op=mybir.AluOpType.add)
                    nc.vector.tensor_scalar(
                        out=rowf[:], in0=rowf[:], scalar1=float(N - 1),
                        scalar2=0.0, op0=mybir.AluOpType.min,
                        op1=mybir.AluOpType.max)
                    rowi = small_pool.tile([1, chunk_w], I32, tag="rowi")
                    nc.vector.tensor_copy(rowi[:], rowf[:])
                    nc.sync.dma_start(
                        out=path_out[chunk * chunk_w:(chunk + 1) * chunk_w],
                        in_=rowi[0:1, :])

    nc.compile()
    return nc


def _get_argmax_nc(n_iters: int = 1, data_bufs: int = DATA_BUFS,
                   mode: str = "full", chunk_w: int = CHUNK):
    key = ("argmax", n_iters, data_bufs, mode, chunk_w)
    if key not in _cache:
        _cache[key] = _build_argmax_nc(n_iters, data_bufs, mode, chunk_w)
    return _cache[key]


def argmax_in_maps(solution_matrix: np.ndarray, chunk_w: int = CHUNK):
    sol = np.ascontiguousarray(solution_matrix)
    nchunk = CPC // chunk_w
    in_maps = []
    for i in range(NCORES):
        shard = sol[:, i * CPC:(i + 1) * CPC]            # [8192, 1024]
        shard = shard.reshape(N, nchunk, chunk_w)
        shard = np.ascontiguousarray(shard.transpose(1, 0, 2))
        in_maps.append({"sol": shard})
    return in_maps


def run_argmax(solution_matrix: np.ndarray, n_iters: int = 1) -> np.ndarray:
    nc = _get_argmax_nc(n_iters)
    res = run_bass_kernel_spmd(nc, argmax_in_maps(solution_matrix),
                               core_ids=list(range(NCORES)))
    path = np.concatenate([res.results[i]["path_shard"] for i in range(NCORES)])
    return path.astype(np.int32)


# ---------------- Launch B: term-sharded gather + sum ----------------

def _build_gather_nc(n_iters: int = 1):
    """SPMD kernel (8 cores): inputs cost [8192, 8192] f32 (replicated),
    pt [1024] i32 = path[j] and nx [1024] i32 = path[(j+1) % n] for this
    core's terms j; output out [1] f32 = sum_j cost[pt[j], nx[j]].

    Terms live at (p, g), j = p*G + g. Each (p, g) fetches the 64-element
    aligned block of row pt containing column nx via one indirect DMA per
    g (base offset = (pt << 13) | (nx & ~63), one descriptor per
    partition), then selects element (nx & 63) with an iota==sel mask and
    reduces everything to a scalar."""
    nc = bacc.Bacc("TRN2", target_bir_lowering=False, debug=False,
                   num_devices=NCORES)
    cost = nc.dram_tensor("cost", [N, N], F32, kind="ExternalInput")
    pt_in = nc.dram_tensor("pt", [CPC], I32, kind="ExternalInput")
    nx_in = nc.dram_tensor("nx", [CPC], I32, kind="ExternalInput")
    out = nc.dram_tensor("out", [1], F32, kind="ExternalOutput")

    cost1 = cost.rearrange("r (k e) -> (r k) e", e=1)  # [N*N, 1]

    with tile.TileContext(nc) as tc:
        with ExitStack() as ctx:
            pool = ctx.enter_context(tc.tile_pool(name="p", bufs=2))
            const_pool = ctx.enter_context(tc.tile_pool(name="c", bufs=1))
            psum_pool = ctx.enter_context(
                tc.tile_pool(name="ps", bufs=2, space="PSUM"))

            # constants: iota [128, B] f32 (0..63 per partition), ones [128,1]
            io_i = const_pool.tile([128, B], I32)
            nc.gpsimd.iota(io_i[:], pattern=[[1, B]], base=0,
                           channel_multiplier=0)
            io_f = const_pool.tile([128, B], F32)
            nc.vector.tensor_copy(io_f[:], io_i[:])
            ones = const_pool.tile([128, 1], F32)
            nc.vector.memset(ones[:], 1.0)

            import contextlib
            loop_cm = (tc.For_i(0, n_iters, 1) if n_iters > 1
                       else contextlib.nullcontext())
            with loop_cm:
                pt = pool.tile([128, G], I32, tag="pt")
                nc.sync.dma_start(
                    out=pt[:], in_=pt_in.rearrange("(p g) -> p g", g=G))
                nx = pool.tile([128, G], I32, tag="nx")
                nc.sync.dma_start(
                    out=nx[:], in_=nx_in.rearrange("(p g) -> p g", g=G))

                # base = (pt << 13) | (nx & ~63); sel = nx & 63
                base = pool.tile([128, G], I32, tag="base")
                nc.vector.tensor_scalar(
                    out=base[:], in0=pt[:], scalar1=13, scalar2=None,
                    op0=mybir.AluOpType.logical_shift_left)
                hi = pool.tile([128, G], I32, tag="hi")
                nc.vector.tensor_scalar(
                    out=hi[:], in0=nx[:], scalar1=N - B, scalar2=None,
                    op0=mybir.AluOpType.bitwise_and)
                nc.vector.tensor_tensor(
                    out=base[:], in0=base[:], in1=hi[:],
                    op=mybir.AluOpType.bitwise_or)
                sel_i = pool.tile([128, G], I32, tag="sel_i")
                nc.vector.tensor_scalar(
                    out=sel_i[:], in0=nx[:], scalar1=B - 1, scalar2=None,
                    op0=mybir.AluOpType.bitwise_and)
                sel_f = pool.tile([128, G], F32, tag="sel_f")
                nc.vector.tensor_copy(sel_f[:], sel_i[:])

                # block gathers: blk[p, g, :] = cost1[base[p, g] .. +B-1]
                blk = pool.tile([128, G, B], F32, tag="blk")
                for g in range(G):
                    nc.gpsimd.indirect_dma_start(
                        out=blk[:, g, :], out_offset=None,
                        in_=cost1[:, :],
                        in_offset=bass.IndirectOffsetOnAxis(
                            ap=base[:, g:g + 1], axis=0))

                # m = (iota == sel); s1 = sum(m * blk) per partition
                m = pool.tile([128, G, B], F32, tag="m")
                io_b = bass.AP(
                    tensor=io_f.tensor, offset=io_f[:].offset,
                    ap=[io_f[:].ap[0], [0, G], io_f[:].ap[1]])
                sel_b = bass.AP(
                    tensor=sel_f.tensor, offset=sel_f[:].offset,
                    ap=[sel_f[:].ap[0], sel_f[:].ap[1], [0, B]])
                nc.vector.tensor_tensor(
                    out=m[:], in0=io_b, in1=sel_b,
                    op=mybir.AluOpType.is_equal)
                scr = pool.tile([128, G, B], F32, tag="scr")
                nc.vector.tensor_tensor(
                    out=scr[:], in0=m[:], in1=blk[:],
                    op=mybir.AluOpType.mult)
                s1 = pool.tile([128, 1], F32, tag="s1")
                nc.vector.reduce_sum(s1[:], scr[:],
                                     axis=mybir.AxisListType.XY)

                # cross-partition sum via ones-vector matmul
                pss = psum_pool.tile([1, 1], F32, tag="pss")
                nc.tensor.matmul(pss[:], ones[:], s1[:], start=True, stop=True)
                so = pool.tile([1, 1], F32, tag="so")
                nc.vector.tensor_copy(so[:], pss[:])
                nc.sync.dma_start(out=out[0:1], in_=so[0:1, 0])

    nc.compile()
    return nc


def _get_gather_nc(n_iters: int = 1):
    key = ("gather", n_iters)
    if key not in _cache:
        _cache[key] = _build_gather_nc(n_iters)
    return _cache[key]


def gather_in_maps(cost_matrix: np.ndarray, path: np.ndarray):
    cost = np.ascontiguousarray(cost_matrix)
    path = np.ascontiguousarray(path.astype(np.int32))
    nxt = np.roll(path, -1)
    return [{"cost": cost,
             "pt": path[i * CPC:(i + 1) * CPC],
             "nx": nxt[i * CPC:(i + 1) * CPC]}
            for i in range(NCORES)]


def run_gather(cost_matrix: np.ndarray, path: np.ndarray,
               n_iters: int = 1) -> np.ndarray:
    nc = _get_gather_nc(n_iters)
    res = run_bass_kernel_spmd(
        nc, gather_in_maps(cost_matrix, path),
        core_ids=list(range(NCORES)))
    total = np.float32(0.0)
    for i in range(NCORES):
        total += np.asarray(res.results[i]["out"], dtype=np.float32)[0]
    return np.asarray([total], dtype=np.float32)


def kernel(solution_matrix: np.ndarray, cost_matrix: np.ndarray) -> np.ndarray:
    path = run_argmax(solution_matrix)
    cost = run_gather(cost_matrix, path)
    return cost


if __name__ == "__main__":
    rng = np.random.default_rng(0)
    sol = rng.standard_normal((N, N), dtype=np.float32)
    cm = rng.random((N, N), dtype=np.float32)
    path = run_argmax(sol)
    want = sol.argmax(axis=0)
    print("argmax match:", np.array_equal(path, want),
          (path != want).sum(), "mismatches")
    got = run_gather(cm, path)
    exp = cm[want, np.roll(want, -1)].sum()
    print("gather:", got, "expect:", exp)
